# revision 52
# baseline (speedup 1.0000x reference)
"""Causal multi-head flash-attention block (QKV proj + attention + out proj)
for Trainium2, distributed over 8 NeuronCores.

Sharding: data-parallel over batch (B=4) x tensor-parallel over head groups
(16 heads -> 2 groups of 8). Core c handles batch c//2, head group c%2.
Each core computes a partial output projection (its 8 heads' contribution);
the host sums the two partials per batch and adds the bias.

v2 reschedule: v1 spent ~235us of its 374us wall with the PE clock-gated
to 1.2 GHz (TRN2 HAM throttles after any idle in a ~3.4us window; the
attention phases' micro-stalls kept re-throttling it).  v2 keeps the PE
stream gap-free:
  - input DMAs emitted in first-use order; warm-up matmuls on the mask
    tile bridge the initial DMA window.
  - all non-attention GEMM work (V strips, next pair's QT/KT projection,
    output projection) is chopped into 2-matmul "filler quanta" and woven
    between attention score/AV groups, so the PE never waits on the
    exp (ACT) pipeline and the HAM stays at 8/8.
  - softmax-normalize chain reworked: DVE reciprocal in-place at
    partition 64, DMA partition-shift of [1,512] to partition 0, Pool
    broadcast, DVE multiply.  (v1 staged via a ~1.9us GpSimd copy per
    tile - 62us of Pool time.)
  - diagonal-group exps are issued per k-block on the causally live
    columns only (also removes all stale-PSUM reads).
  - out-proj PSUM evacuations go to the Scalar engine (idle at the tail;
    Copy shares the Exp activation table so no table reloads).

Per-core kernel (all matmuls bf16 operands, fp32 PSUM accumulate):
  - QKV proj from host-pretransposed x^T: Q^T,K^T in [d, s] layout, V in
    [s, d] layout with a ones-column per head (rowsum trick).
  - Scores computed transposed ST[k,q] via lhsT=KT-block, rhs=QT; two
    heads packed in the PE array via row tiling (contraction K=64 each,
    partitions 0:64 / 64:128).
  - softmax without max-subtraction (logits ~ N(0,1)); exp on ACT with
    the 1/8 scale folded in; causal masking by 0/1 mask multiply
    post-exp on the diagonal blocks.
  - AV: lhsT = V-tile [128, 65] (65th col = ones -> row 64 of PSUM
    accumulates the softmax denominator), rhs = P^T tiles; emitted two
    k-groups behind scores (software pipeline).
  - Output proj from O^T [head*64+d, s] chunks against w_proj rows.
"""

from collections import deque

import numpy as np
import ml_dtypes

import concourse.bass as bass
import concourse.bacc as bacc
import concourse.mybir as mybir
import concourse.tile as tile
from concourse.bass_utils import run_bass_kernel_spmd

F32 = mybir.dt.float32
BF16 = mybir.dt.bfloat16
EXP = mybir.ActivationFunctionType.Exp

# Problem constants (hardcoded per contract)
B, S, C = 4, 2048, 1024
NH, D = 16, 64
SCALE = D ** -0.5
N_CORES = 8
HG = NH // 2          # heads per core (head group)
NPAIR = HG // 2       # head pairs per core
CCH = C // 128        # contraction chunks for QKV proj
SC = S // 128         # s-chunks (also k-blocks count)
NQT = S // 512        # q-tiles of 512
GW = C // 2           # group width of qkv output (8 heads * 64)

WARMUP_MM = 32        # HAM warm-up matmuls during the initial DMA window
PULL = {0: 7, 1: 2, 2: 2, 3: 8}   # filler quanta consumed per score group


def build_nc():
    nc = bacc.Bacc("TRN2", target_bir_lowering=False, debug=False)

    xT = nc.dram_tensor("xT", [C, S], BF16, kind="ExternalInput")
    wq = nc.dram_tensor("wq", [C, GW], BF16, kind="ExternalInput")
    wk = nc.dram_tensor("wk", [C, GW], BF16, kind="ExternalInput")
    wv = nc.dram_tensor("wv", [C, GW], BF16, kind="ExternalInput")
    wp = nc.dram_tensor("wp", [GW, C], BF16, kind="ExternalInput")
    mask = nc.dram_tensor("mask", [128, 512], BF16, kind="ExternalInput")
    out = nc.dram_tensor("out", [S, C], F32, kind="ExternalOutput")

    with tile.TileContext(nc) as tc:
        with (
            tc.tile_pool(name="const", bufs=1) as cpool,
            tc.tile_pool(name="qk", bufs=2) as qkpool,
            tc.tile_pool(name="pt", bufs=10) as ptpool,
            tc.tile_pool(name="work", bufs=2) as wpool,
            tc.tile_pool(name="sps", bufs=2, space="PSUM") as spspool,
            tc.tile_pool(name="mm", bufs=2, space="PSUM") as mmpool,
            tc.tile_pool(name="otp", bufs=2, space="PSUM") as otpool,
        ):
            # ---- input DMAs in first-use order ----
            mask_sb = cpool.tile([128, 512], BF16, tag="mask", name="maskt")
            nc.sync.dma_start(mask_sb[:], mask[:, :])
            xt_sb, wq_sb, wk_sb, wv_sb = [], [], [], []
            for cc in range(CCH):
                t = cpool.tile([128, S], BF16, tag=f"xt{cc}", name=f"xt{cc}")
                nc.sync.dma_start(t[:], xT[128 * cc:128 * (cc + 1), :])
                xt_sb.append(t)
                for name, dram, lst in (("wq", wq, wq_sb), ("wk", wk, wk_sb),
                                        ("wv", wv, wv_sb)):
                    t = cpool.tile([128, GW], BF16, tag=f"{name}{cc}",
                                   name=f"{name}{cc}")
                    nc.sync.dma_start(t[:], dram[128 * cc:128 * (cc + 1), :])
                    lst.append(t)
            wp_sb = []
            for p in range(NPAIR):
                t = cpool.tile([128, C], BF16, tag=f"wp{p}", name=f"wp{p}")
                nc.sync.dma_start(t[:], wp[128 * p:128 * (p + 1), :])
                wp_sb.append(t)
            # preload the ACT exp table set while input DMAs run
            actwarm = cpool.tile([1, 8], F32, tag="actwarm", name="actwarm")
            nc.vector.memset(actwarm[:], 0.0)
            nc.scalar.activation(actwarm[:], actwarm[:], EXP)

            # ---- HAM warm-up: dummy matmuls on a locally memset tile (no
            # DMA dependency, so the PE is busy from ~1us) ----
            warm_src = cpool.tile([128, 512], BF16, tag="warmsrc",
                                  name="warmsrc")
            nc.gpsimd.memset(warm_src[:], 0.5)
            warm_ps = otpool.tile([128, 512], F32, tag="ot", name="warm")
            for _ in range(WARMUP_MM):
                nc.tensor.matmul(warm_ps[:], warm_src[:, 0:128],
                                 warm_src[:, 0:512], start=True, stop=True)

            # O^T normalized, per head pair: head0 partitions 0:64,
            # head1 partitions 64:128 (layout = rows of w_proj)
            otn_sb = [cpool.tile([128, S], BF16, tag=f"otn{p}", name=f"otn{p}")
                      for p in range(NPAIR)]
            vt_sb = [cpool.tile([128, 65 * HG], BF16, tag=f"vt{sc}",
                                name=f"vt{sc}")
                     for sc in range(SC)]

            # ---- filler machinery: (closure, v_strip_done_marker) ----
            fillers = deque()
            v_done = [0]         # strips fully emitted (evac included)

            def pull(n):
                for _ in range(min(n, len(fillers))):
                    fn, marker = fillers.popleft()
                    fn()
                    if marker is not None:
                        v_done[0] = marker + 1

            def ensure_v(sc):
                while v_done[0] <= sc:
                    fn, marker = fillers.popleft()
                    fn()
                    if marker is not None:
                        v_done[0] = marker + 1

            def make_v_quanta(sc):
                st = {}

                def q_memset(sc=sc):
                    nc.gpsimd.memset(vt_sb[sc][:], 1.0)

                def q_mm(i, sc=sc):
                    def f():
                        if i == 0:
                            st["ps"] = mmpool.tile([128, GW], F32, tag="mm",
                                                   name="vps")
                        ps = st["ps"]
                        for cc in (2 * i, 2 * i + 1):
                            nc.tensor.matmul(
                                ps[:], xt_sb[cc][:, 128 * sc:128 * (sc + 1)],
                                wv_sb[cc][:], start=(cc == 0),
                                stop=(cc == CCH - 1))
                    return f

                def q_evac(sc=sc):
                    ps = st["ps"]
                    vt_v = vt_sb[sc][:, :].rearrange(
                        "p (h d) -> p h d", h=HG)[:, :, 0:64]
                    ps_v = ps[:, :].rearrange("p (h d) -> p h d", h=HG)
                    nc.vector.tensor_copy(vt_v, ps_v)

                return ([(q_memset, None)]
                        + [(q_mm(i), None) for i in range(4)]
                        + [(q_evac, sc)])

            qts, kts = {}, {}

            def make_qk_quanta(p, evac_engine="v"):
                """QT/KT projection for pair p as a list of quanta."""
                qt = qkpool.tile([128, S], BF16, tag="qt", name=f"qt{p}")
                kt = qkpool.tile([128, S], BF16, tag="kt", name=f"kt{p}")
                qts[p], kts[p] = qt, kt
                quanta = []
                for st_i in range(NQT):
                    ssl = slice(512 * st_i, 512 * (st_i + 1))
                    for w_sb, dst, eng in ((wq_sb, qt, "v"),
                                           (wk_sb, kt, evac_engine)):
                        box = {}

                        def q_mm(i, box=box, w_sb=w_sb, ssl=ssl, p=p):
                            def f():
                                if i == 0:
                                    box["ps"] = mmpool.tile(
                                        [128, 512], F32, tag="mm", name="qkps")
                                ps = box["ps"]
                                for cc in (2 * i, 2 * i + 1):
                                    nc.tensor.matmul(
                                        ps[:],
                                        w_sb[cc][:, 128 * p:128 * (p + 1)],
                                        xt_sb[cc][:, ssl],
                                        start=(cc == 0), stop=(cc == CCH - 1))
                            return f

                        def q_evac(box=box, dst=dst, ssl=ssl, eng=eng):
                            if eng == "s":
                                nc.scalar.copy(dst[:, ssl], box["ps"][:])
                            else:
                                nc.vector.tensor_copy(dst[:, ssl], box["ps"][:])

                        quanta += [(q_mm(i), None) for i in range(4)]
                        quanta.append((q_evac, None))
                return quanta

            def make_outproj_quanta(sc, evac_eng="v"):
                box = {}

                def q_mm(half, i, sc=sc, box=box):
                    def f():
                        if half == 0 and i == 0:
                            box["outst"] = wpool.tile([128, C], F32,
                                                      tag="outst", name="outst")
                        if i == 0:
                            box["pp"] = mmpool.tile([128, 512], F32, tag="mm",
                                                    name="pp")
                        pp = box["pp"]
                        for p in (2 * i, 2 * i + 1):
                            nc.tensor.matmul(
                                pp[:], otn_sb[p][:, 128 * sc:128 * (sc + 1)],
                                wp_sb[p][:, 512 * half:512 * (half + 1)],
                                start=(p == 0), stop=(p == NPAIR - 1))
                    return f

                def q_evac(half, box=box, eng=evac_eng):
                    def f():
                        if eng == "s":
                            nc.scalar.copy(
                                box["outst"][:, 512 * half:512 * (half + 1)],
                                box["pp"][:])
                        else:
                            nc.vector.tensor_copy(
                                box["outst"][:, 512 * half:512 * (half + 1)],
                                box["pp"][:])
                    return f

                def q_dma(sc=sc, box=box):
                    nc.sync.dma_start(out[128 * sc:128 * (sc + 1), :],
                                      box["outst"][:])

                quanta = []
                for half in range(2):
                    quanta += [(q_mm(half, 0), None), (q_mm(half, 1), None),
                               (q_evac(half), None)]
                quanta.append((q_dma, None))
                return quanta

            def make_scratch_quanta(n):
                # HAM-keepalive: dummy matmuls for filler-dry stretches
                quanta = []
                for _ in range(n):
                    def q():
                        ps = mmpool.tile([128, 512], F32, tag="mm",
                                         name="scr")
                        for _ in range(2):
                            nc.tensor.matmul(ps[:], warm_src[:, 0:128],
                                             warm_src[:, 0:512],
                                             start=True, stop=True)
                    quanta.append((q, None))
                return quanta

            # ---- phase A: pair-0 QT/KT projection (paced by input DMA),
            # then the first V strips (these overlap the DMA tail) ----
            for fn, _ in make_qk_quanta(0, evac_engine="s"):
                fn()
            for sc in range(4):
                for fn, _ in make_v_quanta(sc):
                    fn()
            v_done[0] = 4

            # ---- attention pairs, with filler interleave ----
            for p in range(NPAIR):
                qt, kt = qts[p], kts[p]
                if p == 0:
                    for sc in range(4, SC):
                        fillers.extend(make_v_quanta(sc))
                if p + 1 < NPAIR:
                    fillers.extend(make_qk_quanta(p + 1))
                if p == NPAIR - 1:
                    # no QK/V work remains; keep the PE warm through the
                    # first (largest) q-tile until out-proj work unlocks
                    fillers.extend(make_scratch_quanta(10))

                # last pair runs its q-tiles largest-first so the final
                # normalize->outproj dependency chain is the shortest one
                j_order = range(NQT) if p + 1 < NPAIR else range(NQT - 1, -1, -1)
                for j in j_order:
                    nkb = 4 * (j + 1)  # causal: only k-blocks 0..nkb-1
                    ot = [otpool.tile([65, 512], F32, tag="ot", name="ot")
                          for _ in range(2)]

                    def emit_av(g, pts, j=j, nkb=nkb, ot=ot, p=p):
                        if p == 0:
                            ensure_v(2 * g + 1)
                        # AV accumulation (65th row = softmax denominator)
                        for kb in (2 * g, 2 * g + 1):
                            o = 128 * (kb - 4 * j) if kb >= 4 * j else 0
                            for h in range(2):
                                nc.tensor.matmul(
                                    ot[h][:, o:512],
                                    vt_sb[kb][:, 65 * (2 * p + h):
                                              65 * (2 * p + h) + 65],
                                    pts[kb % 2][:, 512 * h + o:
                                                512 * (h + 1)],
                                    start=(kb == 0), stop=(kb == nkb - 1))

                    pending = []
                    for g in range(nkb // 2):
                        # scores (transposed): per k-block one PSUM tile
                        # holding BOTH heads (cols 0:512 h0, 512:1024 h1)
                        # so the two row-tiled matmuls share one slot wait
                        # and issue back-to-back (concurrent in the PE).
                        # Diagonal blocks restrict to the causally live
                        # columns [o:512]; their exps read the same
                        # restriction, so no PSUM column is ever read
                        # unwritten.
                        pts = []
                        for kb in (2 * g, 2 * g + 1):
                            o = 128 * (kb - 4 * j) if kb >= 4 * j else 0
                            sp = spspool.tile([128, 1024], F32, tag="sps",
                                              name="sps")
                            for h in range(2):
                                hsl = slice(64 * h, 64 * (h + 1))
                                nc.tensor.matmul(
                                    sp[:, 512 * h + o:512 * (h + 1)],
                                    kt[hsl, 128 * kb:128 * (kb + 1)],
                                    qt[hsl, 512 * j + o:512 * (j + 1)],
                                    start=True, stop=True)
                            pt = ptpool.tile([128, 1024], BF16, tag="pt",
                                             name="pt")
                            if o == 0:
                                nc.scalar.activation(pt[:], sp[:], EXP,
                                                     scale=SCALE)
                            else:
                                ptv = pt[:, :].rearrange(
                                    "p (h q) -> p h q", h=2)[:, :, o:512]
                                spv = sp[:, :].rearrange(
                                    "p (h q) -> p h q", h=2)[:, :, o:512]
                                nc.scalar.activation(ptv, spv, EXP,
                                                     scale=SCALE)
                            # causal mask on diagonal blocks (multiplicative)
                            if kb >= 4 * j:
                                for h in range(2):
                                    nc.vector.tensor_mul(
                                        pt[:, 512 * h + o:512 * (h + 1)],
                                        pt[:, 512 * h + o:512 * (h + 1)],
                                        mask_sb[:, 0:512 - o])
                            pts.append(pt)
                        pending.append((g, pts))
                        if len(pending) > 3:
                            # retire TWO groups back-to-back: the PE pays
                            # ~107ns on every matmul whose shape differs
                            # from its predecessor (measured rate 1.00), so
                            # batching 8 same-shape AV matmuls halves the
                            # AV-block transition count
                            emit_av(*pending.pop(0))
                            emit_av(*pending.pop(0))
                        pull(PULL[p])
                    for item in pending:
                        emit_av(*item)
                        pull(2)

                    # ---- normalize O^T chunk by the softmax denominator.
                    # ot row 64 holds the denominators; a [1,512] DMA
                    # shifts that row to partition 0 (custom-DVE ops and
                    # partition_broadcast both want partition 0), DVE
                    # takes the reciprocal, Pool broadcasts, DVE
                    # multiplies.
                    qsl = slice(512 * j, 512 * (j + 1))
                    for h in range(2):
                        s64 = wpool.tile([65, 512], F32, tag="s64",
                                         name="s64")
                        nc.vector.tensor_copy(s64[:, 0:512], ot[h][:, :])
                        bsrc = wpool.tile([1, 512], F32, tag="bsrc",
                                          name="bsrc")
                        nc.sync.dma_start(bsrc[0:1, :], s64[64:65, 0:512])
                        rinv = wpool.tile([1, 512], F32, tag="rinv",
                                          name="rinv")
                        nc.vector.reciprocal_approx_fast(rinv[0:1, :],
                                                         bsrc[0:1, :])
                        bcs = wpool.tile([64, 512], F32, tag="bcs", name="bcs")
                        nc.gpsimd.partition_broadcast(bcs[:], rinv[0:1, :])
                        if h == 0:
                            nc.vector.tensor_mul(otn_sb[p][0:64, qsl],
                                                 s64[0:64, 0:512], bcs[:])
                        else:
                            oth = wpool.tile([64, 512], BF16, tag="oth",
                                             name="oth")
                            nc.vector.tensor_mul(oth[:], s64[0:64, 0:512],
                                                 bcs[:])
                            # partition-shifting copy into rows 64:128
                            nc.sync.dma_start(otn_sb[p][64:128, qsl], oth[:])

                    if p == NPAIR - 1:
                        # j>=2 chunks flow as fillers while exps still run
                        # (DVE evac); j<=1 chunks land in the tail where the
                        # Scalar engine is idle (ACT evac)
                        for sc in range(4 * j, 4 * (j + 1)):
                            fillers.extend(make_outproj_quanta(
                                sc, evac_eng="v" if j >= 2 else "s"))

                # pair boundary: QT/KT of p+1 (and pair-0's V strips) must
                # be complete before the next pair's scores
                if p + 1 < NPAIR:
                    pull(len(fillers))

            # tail: remaining output-projection chunks
            pull(len(fillers))

    nc.compile()
    return nc


_NC_CACHE = None


def _get_nc():
    global _NC_CACHE
    if _NC_CACHE is None:
        _NC_CACHE = build_nc()
    return _NC_CACHE


def make_in_maps(x, w_qkv, w_proj):
    """Shard full inputs into the 8 per-core input dicts."""
    bf = ml_dtypes.bfloat16
    mask01 = (np.arange(128)[:, None] <= np.arange(512)[None, :]) \
        .astype(bf)
    in_maps = []
    for core in range(N_CORES):
        b, g = core // 2, core % 2
        gsl = slice(GW * g, GW * (g + 1))
        in_maps.append({
            "xT": np.ascontiguousarray(x[b].T).astype(bf),
            "wq": np.ascontiguousarray(w_qkv[:, 0 * C:1 * C][:, gsl]).astype(bf),
            "wk": np.ascontiguousarray(w_qkv[:, 1 * C:2 * C][:, gsl]).astype(bf),
            "wv": np.ascontiguousarray(w_qkv[:, 2 * C:3 * C][:, gsl]).astype(bf),
            "wp": np.ascontiguousarray(w_proj[gsl, :]).astype(bf),
            "mask": mask01,
        })
    return in_maps


def kernel(x, w_qkv, w_proj, b_proj, _profile=False):
    import os
    if not _profile:
        # the NTFF trace path needs modules absent from this image;
        # make sure an inherited BASS_TRACE can't route us into it
        os.environ["BASS_NEVER_TRACE"] = "1"
    else:
        os.environ.pop("BASS_NEVER_TRACE", None)
    x = np.asarray(x, np.float32)
    w_qkv = np.asarray(w_qkv, np.float32)
    w_proj = np.asarray(w_proj, np.float32)
    b_proj = np.asarray(b_proj, np.float32)

    nc = _get_nc()
    in_maps = make_in_maps(x, w_qkv, w_proj)
    res = run_bass_kernel_spmd(nc, in_maps, core_ids=list(range(N_CORES)),
                               trace=_profile)
    partials = [res.results[c]["out"] for c in range(N_CORES)]
    out = np.empty((B, S, C), np.float32)
    for b in range(B):
        out[b] = partials[2 * b] + partials[2 * b + 1] + b_proj
    if _profile:
        return out, res
    return out
